# revision 5
# baseline (speedup 1.0000x reference)
"""Kronecker layer forward on 8 TRN2 NeuronCores.

Computes y = gelu_exact(x @ kron(B, A)) + bias for
  x [16384, 4096] f32, A [64, 64], B [64, 64], bias [4096].

Math: with x3 = x.reshape(n, 64, 64) (feature f = i*64 + k),
  y[b, j*64+l] = sum_{i,k} x3[b,i,k] * B[i,j] * A[k,l].

Per 4 tokens {base+2g+h : g,h in {0,1}} we form one 128x128 SBUF tile
  xt[(g,i), (h,k)] = x[base+2g+h, i*64+k]
and chain two TensorE matmuls with the DATA as the stationary operand and
block-diagonal replicated factors as the 128-col moving operand:
  out1 = xt.T  @ blockdiag(B,B)   -> [(h,k), (g,j)]   (contracts i)
  out2 = out1.T @ blockdiag(A,A)  -> [(g,j), (h,l)]   (contracts k)
out2 is exactly the y-layout view y[base+2g+h, j*64+l] — both contraction
dims land on partitions with zero transposes, and all DMA access patterns
have 256B contiguous bursts.

Sharding: pure data-parallel over the token dim — 2048 tokens per core,
A/B/bias replicated, no collectives.
"""

import numpy as np

N_CORES = 8
TOKENS = 16384
D = 4096
TPC = TOKENS // N_CORES  # tokens per core

_CACHE = {}


def _build(tpc, mm_dtype, with_bias, n_cores):
    import concourse.bacc as bacc
    import concourse.mybir as mybir
    import concourse.tile as tile

    f32 = mybir.dt.float32
    mmdt = getattr(mybir.dt, mm_dtype)

    nc = bacc.Bacc(
        "TRN2",
        target_bir_lowering=False,
        debug=False,
        num_devices=n_cores,
    )
    x_d = nc.dram_tensor("x", [tpc, D], mmdt, kind="ExternalInput").ap()
    bd_d = nc.dram_tensor("bd", [128, 128], mmdt, kind="ExternalInput").ap()
    ad_d = nc.dram_tensor("ad", [128, 128], mmdt, kind="ExternalInput").ap()
    if with_bias:
        bias_d = nc.dram_tensor("bias_t", [128, 128], f32, kind="ExternalInput").ap()
    y_d = nc.dram_tensor("y", [tpc, D], f32, kind="ExternalOutput").ap()

    nsuper = tpc // 4
    # [nb, g, i, h, k]: per-supertile source for the [(g,i), (h,k)] SBUF tile
    xv = x_d.rearrange(
        "(nb g h) (i k) -> nb g h i k", g=2, h=2, k=64
    ).transpose([0, 1, 3, 2, 4])
    # [nb, g, j, h, l]: per-supertile dest from the [(g,j), (h,l)] SBUF tile
    yv = y_d.rearrange(
        "(nb g h) (j l) -> nb g h j l", g=2, h=2, l=64
    ).transpose([0, 1, 3, 2, 4])

    with tile.TileContext(nc) as tc:
        with (
            tc.tile_pool(name="const", bufs=1) as constp,
            tc.tile_pool(name="xp", bufs=4) as xp,
            tc.tile_pool(name="up", bufs=4) as up,
            tc.tile_pool(name="yp", bufs=4) as yp,
            tc.tile_pool(name="ps1", bufs=4, space="PSUM") as ps1,
            tc.tile_pool(name="ps2", bufs=4, space="PSUM") as ps2,
        ):
            bd = constp.tile([128, 128], mmdt)
            nc.sync.dma_start(bd[:], bd_d)
            ad = constp.tile([128, 128], mmdt)
            nc.sync.dma_start(ad[:], ad_d)
            if with_bias:
                bias_t = constp.tile([128, 128], f32)
                nc.sync.dma_start(bias_t[:], bias_d)

            for nb in range(nsuper):
                xt = xp.tile([128, 128], mmdt)
                nc.sync.dma_start(xt[:64, :], xv[nb, 0])
                nc.sync.dma_start(xt[64:, :], xv[nb, 1])
                o1 = ps1.tile([128, 128], f32)
                nc.tensor.matmul(o1[:], xt[:], bd[:])
                u = up.tile([128, 128], mmdt)
                nc.vector.tensor_copy(u[:], o1[:])
                o2 = ps2.tile([128, 128], f32)
                nc.tensor.matmul(o2[:], u[:], ad[:])
                yt = yp.tile([128, 128], f32)
                nc.scalar.activation(
                    yt[:], o2[:], mybir.ActivationFunctionType.Gelu
                )
                if with_bias:
                    nc.vector.tensor_add(yt[:], yt[:], bias_t[:])
                nc.sync.dma_start(yv[nb, 0], yt[:64, :])
                nc.sync.dma_start(yv[nb, 1], yt[64:, :])

    nc.compile()
    return nc


def _get_nc(tpc, mm_dtype, with_bias, n_cores=N_CORES):
    key = (tpc, mm_dtype, with_bias, n_cores)
    if key not in _CACHE:
        _CACHE[key] = _build(*key)
    return _CACHE[key]


def _run(x, A, B, bias, mm_dtype="float32", tpc=TPC, trace=False):
    from concourse.bass_utils import run_bass_kernel_spmd

    n = x.shape[0]
    n_cores = n // tpc
    assert n == n_cores * tpc

    with_bias = bool(np.any(bias))
    nc = _get_nc(tpc, mm_dtype, with_bias, n_cores)

    np_mmdt = np.float32 if mm_dtype == "float32" else None
    if np_mmdt is None:
        import ml_dtypes

        np_mmdt = ml_dtypes.bfloat16

    Bd = np.zeros((128, 128), np_mmdt)
    Bd[:64, :64] = B.astype(np_mmdt)
    Bd[64:, 64:] = B.astype(np_mmdt)
    Ad = np.zeros((128, 128), np_mmdt)
    Ad[:64, :64] = A.astype(np_mmdt)
    Ad[64:, 64:] = A.astype(np_mmdt)

    in_maps = []
    for c in range(n_cores):
        m = {
            "x": np.ascontiguousarray(x[c * tpc : (c + 1) * tpc]).astype(np_mmdt),
            "bd": Bd,
            "ad": Ad,
        }
        if with_bias:
            # bias_t[(g,j), (h,l)] = bias[j*64 + l]
            bt = np.tile(bias.astype(np.float32).reshape(64, 64), (2, 2))
            m["bias_t"] = np.ascontiguousarray(bt)
        in_maps.append(m)

    res = run_bass_kernel_spmd(
        nc, in_maps, list(range(n_cores)), trace=trace,
        trace_cores=list(range(n_cores)) if trace else None,
    )
    y = np.concatenate([r["y"] for r in res.results], axis=0)
    return y.astype(np.float32), res


def kernel(x, A, B, bias):
    y, _ = _run(
        np.asarray(x), np.asarray(A), np.asarray(B), np.asarray(bias),
        mm_dtype="float32",
    )
    return y


# revision 9
# speedup vs baseline: 2.6363x; 2.6363x over previous
"""Kronecker layer forward on 8 TRN2 NeuronCores.

Computes y = gelu_exact(x @ kron(B, A)) + bias for
  x [16384, 4096] f32, A [64, 64], B [64, 64], bias [4096].

Math: with x3 = x.reshape(n, 64, 64) (feature f = i*64 + k),
  y[b, j*64+l] = sum_{i,k} x3[b,i,k] * B[i,j] * A[k,l].

Per supertile s we pick 4 tokens t(g,h) = g*(tpc/2) + h*(tpc/4) + s
(g,h in {0,1}) and form one 128x128 SBUF tile
  xt[(g,i), (h,k)] = x[t(g,h), i*64+k]
then chain two TensorE matmuls with the DATA as the stationary operand and
a 256-wide concatenated pair of block-diagonal factors as the moving one:
  o1 = xt.T @ [blockdiag(B,B) | blockdiag(A,A)]   left  half: [(h,k),(g,j)]
  o2 = u.T  @ [blockdiag(B,B) | blockdiag(A,A)]   right half: [(g,j),(h,l)]
(u = left half of o1 copied to SBUF). o2's right half is exactly the
y-layout view y[t(g,h), j*64+l] — both contraction dims land on partitions
with zero transposes. The 256-wide moving operand keeps float32r matmuls
at 1 cycle/row (4x faster than fp32) at the cost of one garbage half per
matmul.

The block-token mapping makes every DMA a single 3-dim access pattern:
one block = NB supertiles, loaded/stored with 4 big DMAs each (one per
(g,h) quadrant), 256B contiguous bursts.

Sharding: pure data-parallel over the token dim — 2048 tokens per core,
A/B/bias replicated, no collectives.
"""

import numpy as np

N_CORES = 8
TOKENS = 16384
D = 4096
TPC = TOKENS // N_CORES  # tokens per core

_CACHE = {}


def _build(tpc, mm_impl, with_bias, n_cores):
    import concourse.bacc as bacc
    import concourse.mybir as mybir
    import concourse.tile as tile

    f32 = mybir.dt.float32
    f32r = mybir.dt.float32r
    use_f32r = mm_impl == "f32r"

    nsuper = tpc // 4
    NB = min(32, nsuper)          # supertiles per block
    assert nsuper % NB == 0
    nblocks = nsuper // NB
    GRP = 4                       # supertiles per PSUM pack
    assert NB % GRP == 0
    W = 256 if use_f32r else 128  # matmul moving width / psum cols per st

    nc = bacc.Bacc(
        "TRN2",
        target_bir_lowering=False,
        debug=False,
        num_devices=n_cores,
    )
    mmdt = f32r if use_f32r else f32
    x_d = nc.dram_tensor("x", [tpc, D], mmdt, kind="ExternalInput").ap()
    w_d = nc.dram_tensor("wcat", [128, W], mmdt, kind="ExternalInput").ap()
    if not use_f32r:
        a_d = nc.dram_tensor("acat", [128, 128], f32, kind="ExternalInput").ap()
    if with_bias:
        bias_d = nc.dram_tensor("bias_t", [128, 128], f32, kind="ExternalInput").ap()
    y_d = nc.dram_tensor("y", [tpc, D], f32, kind="ExternalOutput").ap()

    # DRAM views: token t = g*(tpc/2) + h*(tpc/4) + s
    # x[(g h s)][(i k)] -> [g, h, blk, i, s_in_blk, k]
    xv = x_d.rearrange(
        "(g h blk s) (i k) -> g h blk s i k", g=2, h=2, s=NB, k=64
    ).transpose([0, 1, 2, 4, 3, 5])
    yv = y_d.rearrange(
        "(g h blk s) (j l) -> g h blk s j l", g=2, h=2, s=NB, l=64
    ).transpose([0, 1, 2, 4, 3, 5])


    with tile.TileContext(nc) as tc:
        with (
            tc.tile_pool(name="const", bufs=1) as constp,
            tc.tile_pool(name="xp", bufs=3) as xp,
            tc.tile_pool(name="up", bufs=4) as up,
            tc.tile_pool(name="yp", bufs=3) as yp,
            tc.tile_pool(name="ps1", bufs=2, space="PSUM") as ps1,
            tc.tile_pool(name="ps2", bufs=2, space="PSUM") as ps2,
        ):
            wcat = constp.tile([128, W], mmdt)
            nc.sync.dma_start(wcat[:], w_d)
            if not use_f32r:
                acat = constp.tile([128, 128], f32)
                nc.sync.dma_start(acat[:], a_d)
            if with_bias:
                bias_t = constp.tile([128, 128], f32)
                nc.sync.dma_start(bias_t[:], bias_d)

            for blk in range(nblocks):
                xbig = xp.tile([128, NB * 128], mmdt)
                ybig = yp.tile([128, NB * 128], f32)
                # [128, NB, 2, 64] views of the SBUF tiles
                xbv = xbig[:].rearrange("p (s h k) -> p s h k", h=2, k=64)
                ybv = ybig[:].rearrange("p (s h l) -> p s h l", h=2, l=64)
                for g in range(2):
                    for h in range(2):
                        nc.sync.dma_start(
                            xbv[g * 64 : (g + 1) * 64, :, h, :],
                            xv[g, h, blk],
                        )

                for grp in range(NB // GRP):
                    o1 = ps1.tile([128, GRP * W], f32)
                    o2 = ps2.tile([128, GRP * W], f32)
                    u = up.tile([128, GRP * 128], f32r if use_f32r else f32)
                    for q in range(GRP):
                        s = grp * GRP + q
                        nc.tensor.matmul(
                            o1[:, q * W : (q + 1) * W],
                            xbig[:, s * 128 : (s + 1) * 128],
                            wcat[:],
                        )
                    # useful half of o1 (cols [0,128) of each W-slot) -> u
                    if use_f32r:
                        o1u = o1[:].rearrange(
                            "p (q two f) -> p q two f", two=2, f=128
                        )[:, :, 0, :]
                    else:
                        o1u = o1[:].rearrange("p (q f) -> p q f", f=128)
                    nc.vector.tensor_copy(
                        u[:].rearrange("p (q f) -> p q f", f=128), o1u
                    )
                    for q in range(GRP):
                        nc.tensor.matmul(
                            o2[:, q * W : (q + 1) * W],
                            u[:, q * 128 : (q + 1) * 128],
                            wcat[:] if use_f32r else acat[:],
                        )
                    # useful half of o2 (right half of each W-slot for f32r)
                    if use_f32r:
                        o2u = o2[:].rearrange(
                            "p (q two f) -> p q two f", two=2, f=128
                        )[:, :, 1, :]
                    else:
                        o2u = o2[:].rearrange("p (q f) -> p q f", f=128)
                    ydst = ybig[:, grp * 512 : (grp + 1) * 512].rearrange(
                        "p (q f) -> p q f", f=128
                    )
                    nc.scalar.activation(
                        ydst, o2u, mybir.ActivationFunctionType.Gelu
                    )
                    if with_bias:
                        bseg = ybig[:, grp * 512 : (grp + 1) * 512].rearrange(
                            "p (q f) -> p q f", f=128
                        )
                        bsrc = bias_t[:].unsqueeze(1).broadcast_to([128, GRP, 128])
                        nc.vector.tensor_add(bseg, bseg, bsrc)

                for g in range(2):
                    for h in range(2):
                        nc.sync.dma_start(
                            yv[g, h, blk],
                            ybv[g * 64 : (g + 1) * 64, :, h, :],
                        )

    nc.compile()
    return nc


def _get_nc(tpc, mm_impl, with_bias, n_cores=N_CORES):
    key = (tpc, mm_impl, with_bias, n_cores)
    if key not in _CACHE:
        _CACHE[key] = _build(*key)
    return _CACHE[key]


def _make_weights(A, B, mm_impl):
    Bd = np.zeros((128, 128), np.float32)
    Bd[:64, :64] = B
    Bd[64:, 64:] = B
    Ad = np.zeros((128, 128), np.float32)
    Ad[:64, :64] = A
    Ad[64:, 64:] = A
    if mm_impl == "f32r":
        return {"wcat": np.ascontiguousarray(np.concatenate([Bd, Ad], axis=1))}
    return {"wcat": Bd, "acat": Ad}


def _shard_perm(tpc):
    # per-core token order is identity: x rows [c*tpc:(c+1)*tpc] feed the
    # view (g h blk s); the view itself encodes t = g*tpc/2 + h*tpc/4 + s.
    return None


def _run(x, A, B, bias, mm_impl="f32r", tpc=TPC, trace=False):
    from concourse.bass_utils import run_bass_kernel_spmd

    n = x.shape[0]
    n_cores = n // tpc
    assert n == n_cores * tpc

    with_bias = bool(np.any(bias))
    nc = _get_nc(tpc, mm_impl, with_bias, n_cores)
    wmaps = _make_weights(np.asarray(A, np.float32), np.asarray(B, np.float32), mm_impl)

    in_maps = []
    for c in range(n_cores):
        m = {"x": np.ascontiguousarray(x[c * tpc : (c + 1) * tpc], dtype=np.float32)}
        m.update(wmaps)
        if with_bias:
            m["bias_t"] = np.ascontiguousarray(
                np.tile(bias.astype(np.float32).reshape(64, 64), (2, 2))
            )
        in_maps.append(m)

    res = run_bass_kernel_spmd(
        nc, in_maps, list(range(n_cores)), trace=trace,
        trace_cores=list(range(n_cores)) if trace else None,
    )
    y = np.concatenate([r["y"] for r in res.results], axis=0)
    return y.astype(np.float32), res


def kernel(x, A, B, bias):
    y, _ = _run(
        np.asarray(x), np.asarray(A), np.asarray(B), np.asarray(bias),
        mm_impl="f32r",
    )
    return y


# revision 12
# speedup vs baseline: 3.8641x; 1.4658x over previous
"""Kronecker layer forward on 8 TRN2 NeuronCores.

Computes y = gelu_exact(x @ kron(B, A)) + bias for
  x [16384, 4096] f32, A [64, 64], B [64, 64], bias [4096].

Math: with x3 = x.reshape(n, 64, 64) (feature f = i*64 + k),
  y[b, j*64+l] = sum_{i,k} x3[b,i,k] * B[i,j] * A[k,l].

Per supertile s we pick 4 tokens t(g,h) = g*(tpc/2) + h*(tpc/4) + s
(g,h in {0,1}) and form one 128x128 SBUF tile
  xt[(g,i), (h,k)] = x[t(g,h), i*64+k]
then chain two TensorE matmuls with the DATA as the stationary operand and
a 256-wide concatenated pair of block-diagonal factors as the moving one:
  o1 = xt.T @ [blockdiag(B,B) | blockdiag(A,A)]   left  half: [(h,k),(g,j)]
  o2 = u.T  @ [blockdiag(B,B) | blockdiag(A,A)]   right half: [(g,j),(h,l)]
(u = left half of o1 copied to SBUF). o2's right half is exactly the
y-layout view y[t(g,h), j*64+l] — both contraction dims land on partitions
with zero transposes. The 256-wide moving operand keeps float32r matmuls
at 1 cycle/row (4x faster than fp32) at the cost of one garbage half per
matmul.

The block-token mapping makes every DMA a single 3-dim access pattern:
one block = NB supertiles, loaded/stored with 4 big DMAs each (one per
(g,h) quadrant), 256B contiguous bursts.

Sharding: pure data-parallel over the token dim — 2048 tokens per core,
A/B/bias replicated, no collectives.
"""

import numpy as np

N_CORES = 8
TOKENS = 16384
D = 4096
TPC = TOKENS // N_CORES  # tokens per core

_CACHE = {}


def _build(tpc, mm_impl, with_bias, n_cores):
    import concourse.bacc as bacc
    import concourse.mybir as mybir
    import concourse.tile as tile

    f32 = mybir.dt.float32
    f32r = mybir.dt.float32r
    use_f32r = mm_impl == "f32r"

    nsuper = tpc // 4
    NB = min(32, nsuper)          # supertiles per block
    assert nsuper % NB == 0
    nblocks = nsuper // NB
    GRP = 4                       # supertiles per PSUM pack
    assert NB % GRP == 0
    W = 256 if use_f32r else 128  # matmul moving width / psum cols per st

    nc = bacc.Bacc(
        "TRN2",
        target_bir_lowering=False,
        debug=False,
        num_devices=n_cores,
    )
    mmdt = f32r if use_f32r else f32
    x_d = nc.dram_tensor("x", [tpc, D], mmdt, kind="ExternalInput").ap()
    w_d = nc.dram_tensor("wcat", [128, W], mmdt, kind="ExternalInput").ap()
    if not use_f32r:
        a_d = nc.dram_tensor("acat", [128, 128], f32, kind="ExternalInput").ap()
    if with_bias:
        bias_d = nc.dram_tensor("bias_t", [128, 128], f32, kind="ExternalInput").ap()
    y_d = nc.dram_tensor("y", [tpc, D], f32, kind="ExternalOutput").ap()

    # DRAM views: token t = g*(tpc/2) + 2*s + h, so for a block the source
    # rows are 2*NB CONSECUTIVE tokens and the SBUF free line is flat:
    # xbig[(g,i), s*128 + h*64 + k] <- x[g*tpc/2 + blk*2NB + (2s+h), i*64+k]
    # [g, blk, i, r, k] with r = 2s+h
    xv = x_d.rearrange(
        "(g blk r) (i k) -> g blk r i k", g=2, r=2 * NB, k=64
    ).transpose([0, 1, 3, 2, 4])
    yv = y_d.rearrange(
        "(g blk r) (j l) -> g blk r j l", g=2, r=2 * NB, l=64
    ).transpose([0, 1, 3, 2, 4])


    with tile.TileContext(nc) as tc:
        with (
            tc.tile_pool(name="const", bufs=1) as constp,
            tc.tile_pool(name="xp", bufs=3) as xp,
            tc.tile_pool(name="up", bufs=4) as up,
            tc.tile_pool(name="yp", bufs=3) as yp,
            tc.tile_pool(name="ps1", bufs=2, space="PSUM") as ps1,
            tc.tile_pool(name="ps2", bufs=2, space="PSUM") as ps2,
        ):
            wcat = constp.tile([128, W], mmdt)
            nc.sync.dma_start(wcat[:], w_d)
            if not use_f32r:
                acat = constp.tile([128, 128], f32)
                nc.sync.dma_start(acat[:], a_d)
            if with_bias:
                bias_t = constp.tile([128, 128], f32)
                nc.sync.dma_start(bias_t[:], bias_d)

            for blk in range(nblocks):
                xbig = xp.tile([128, NB * 128], mmdt)
                ybig = yp.tile([128, NB * 128], f32)
                for g in range(2):
                    nc.sync.dma_start(
                        xbig[g * 64 : (g + 1) * 64, :], xv[g, blk]
                    )

                for grp in range(NB // GRP):
                    o1 = ps1.tile([128, GRP * W], f32)
                    o2 = ps2.tile([128, GRP * W], f32)
                    u = up.tile([128, GRP * 128], f32r if use_f32r else f32)
                    for q in range(GRP):
                        s = grp * GRP + q
                        nc.tensor.matmul(
                            o1[:, q * W : (q + 1) * W],
                            xbig[:, s * 128 : (s + 1) * 128],
                            wcat[:],
                        )
                    # useful half of o1 (cols [0,128) of each W-slot) -> u
                    if use_f32r:
                        o1u = o1[:].rearrange(
                            "p (q two f) -> p q two f", two=2, f=128
                        )[:, :, 0, :]
                    else:
                        o1u = o1[:].rearrange("p (q f) -> p q f", f=128)
                    nc.vector.tensor_copy(
                        u[:].rearrange("p (q f) -> p q f", f=128), o1u
                    )
                    for q in range(GRP):
                        nc.tensor.matmul(
                            o2[:, q * W : (q + 1) * W],
                            u[:, q * 128 : (q + 1) * 128],
                            wcat[:] if use_f32r else acat[:],
                        )
                    # useful half of o2 (right half of each W-slot for f32r)
                    if use_f32r:
                        o2u = o2[:].rearrange(
                            "p (q two f) -> p q two f", two=2, f=128
                        )[:, :, 1, :]
                    else:
                        o2u = o2[:].rearrange("p (q f) -> p q f", f=128)
                    ydst = ybig[:, grp * 512 : (grp + 1) * 512].rearrange(
                        "p (q f) -> p q f", f=128
                    )
                    nc.scalar.activation(
                        ydst, o2u, mybir.ActivationFunctionType.Gelu
                    )
                    if with_bias:
                        bseg = ybig[:, grp * 512 : (grp + 1) * 512].rearrange(
                            "p (q f) -> p q f", f=128
                        )
                        bsrc = bias_t[:].unsqueeze(1).broadcast_to([128, GRP, 128])
                        nc.vector.tensor_add(bseg, bseg, bsrc)

                for g in range(2):
                    nc.gpsimd.dma_start(
                        yv[g, blk], ybig[g * 64 : (g + 1) * 64, :]
                    )

    nc.compile()
    return nc


def _get_nc(tpc, mm_impl, with_bias, n_cores=N_CORES):
    key = (tpc, mm_impl, with_bias, n_cores)
    if key not in _CACHE:
        _CACHE[key] = _build(*key)
    return _CACHE[key]


def _make_weights(A, B, mm_impl):
    Bd = np.zeros((128, 128), np.float32)
    Bd[:64, :64] = B
    Bd[64:, 64:] = B
    Ad = np.zeros((128, 128), np.float32)
    Ad[:64, :64] = A
    Ad[64:, 64:] = A
    if mm_impl == "f32r":
        return {"wcat": np.ascontiguousarray(np.concatenate([Bd, Ad], axis=1))}
    return {"wcat": Bd, "acat": Ad}


def _shard_perm(tpc):
    # per-core token order is identity: x rows [c*tpc:(c+1)*tpc] feed the
    # view (g h blk s); the view itself encodes t = g*tpc/2 + h*tpc/4 + s.
    return None


def _run(x, A, B, bias, mm_impl="f32r", tpc=TPC, trace=False):
    from concourse.bass_utils import run_bass_kernel_spmd

    n = x.shape[0]
    n_cores = n // tpc
    assert n == n_cores * tpc

    with_bias = bool(np.any(bias))
    nc = _get_nc(tpc, mm_impl, with_bias, n_cores)
    wmaps = _make_weights(np.asarray(A, np.float32), np.asarray(B, np.float32), mm_impl)

    in_maps = []
    for c in range(n_cores):
        m = {"x": np.ascontiguousarray(x[c * tpc : (c + 1) * tpc], dtype=np.float32)}
        m.update(wmaps)
        if with_bias:
            m["bias_t"] = np.ascontiguousarray(
                np.tile(bias.astype(np.float32).reshape(64, 64), (2, 2))
            )
        in_maps.append(m)

    res = run_bass_kernel_spmd(
        nc, in_maps, list(range(n_cores)), trace=trace,
        trace_cores=list(range(n_cores)) if trace else None,
    )
    y = np.concatenate([r["y"] for r in res.results], axis=0)
    return y.astype(np.float32), res


def kernel(x, A, B, bias):
    y, _ = _run(
        np.asarray(x), np.asarray(A), np.asarray(B), np.asarray(bias),
        mm_impl="f32r",
    )
    return y


# revision 13
# speedup vs baseline: 5.7300x; 1.4829x over previous
"""Kronecker layer forward on 8 TRN2 NeuronCores.

Computes y = gelu_exact(x @ kron(B, A)) + bias for
  x [16384, 4096] f32, A [64, 64], B [64, 64], bias [4096].

Math: with x3 = x.reshape(n, 64, 64) (feature f = i*64 + k),
  y[b, j*64+l] = sum_{i,k} x3[b,i,k] * B[i,j] * A[k,l].

Per supertile s we pick 4 tokens t(g,h) = g*(tpc/2) + h*(tpc/4) + s
(g,h in {0,1}) and form one 128x128 SBUF tile
  xt[(g,i), (h,k)] = x[t(g,h), i*64+k]
then chain two TensorE matmuls with the DATA as the stationary operand and
a 256-wide concatenated pair of block-diagonal factors as the moving one:
  o1 = xt.T @ [blockdiag(B,B) | blockdiag(A,A)]   left  half: [(h,k),(g,j)]
  o2 = u.T  @ [blockdiag(B,B) | blockdiag(A,A)]   right half: [(g,j),(h,l)]
(u = left half of o1 copied to SBUF). o2's right half is exactly the
y-layout view y[t(g,h), j*64+l] — both contraction dims land on partitions
with zero transposes. The 256-wide moving operand keeps float32r matmuls
at 1 cycle/row (4x faster than fp32) at the cost of one garbage half per
matmul.

The block-token mapping makes every DMA a single 3-dim access pattern:
one block = NB supertiles, loaded/stored with 4 big DMAs each (one per
(g,h) quadrant), 256B contiguous bursts.

Sharding: pure data-parallel over the token dim — 2048 tokens per core,
A/B/bias replicated, no collectives.
"""

import numpy as np

N_CORES = 8
TOKENS = 16384
D = 4096
TPC = TOKENS // N_CORES  # tokens per core

_CACHE = {}


def _build(tpc, mm_impl, with_bias, n_cores):
    import concourse.bacc as bacc
    import concourse.mybir as mybir
    import concourse.tile as tile

    f32 = mybir.dt.float32
    f32r = mybir.dt.float32r
    use_f32r = mm_impl == "f32r"

    nsuper = tpc // 4
    NB = min(32, nsuper)          # supertiles per block
    assert nsuper % NB == 0
    nblocks = nsuper // NB
    GRP = 4                       # supertiles per PSUM pack
    assert NB % GRP == 0
    W = 256 if use_f32r else 128  # matmul moving width / psum cols per st

    nc = bacc.Bacc(
        "TRN2",
        target_bir_lowering=False,
        debug=False,
        num_devices=n_cores,
    )
    mmdt = f32r if use_f32r else f32
    x_d = nc.dram_tensor(
        "x", [2, nblocks, 64 * 2 * NB * 64], mmdt, kind="ExternalInput"
    ).ap()
    w_d = nc.dram_tensor("wcat", [128, W], mmdt, kind="ExternalInput").ap()
    if not use_f32r:
        a_d = nc.dram_tensor("acat", [128, 128], f32, kind="ExternalInput").ap()
    if with_bias:
        bias_d = nc.dram_tensor("bias_t", [128, 128], f32, kind="ExternalInput").ap()
    y_d = nc.dram_tensor(
        "y", [2, nblocks, 64 * 2 * NB * 64], f32, kind="ExternalOutput"
    ).ap()

    # The host pre-permutes x to [g, blk, i, r, k] (r = 2s+h, token
    # t = g*tpc/2 + blk*2NB + r) so each (g, blk) slab is one fully
    # contiguous 1MB DMA whose element order matches the SBUF tile
    # [p=(g,i), free=r*64+k] exactly. y is written the same way
    # ([g, blk, j, r, l]) and inverse-permuted on the host.


    with tile.TileContext(nc) as tc:
        with (
            tc.tile_pool(name="const", bufs=1) as constp,
            tc.tile_pool(name="xp", bufs=3) as xp,
            tc.tile_pool(name="up", bufs=4) as up,
            tc.tile_pool(name="yp", bufs=3) as yp,
            tc.tile_pool(name="ps1", bufs=2, space="PSUM") as ps1,
            tc.tile_pool(name="ps2", bufs=2, space="PSUM") as ps2,
        ):
            wcat = constp.tile([128, W], mmdt)
            nc.sync.dma_start(wcat[:], w_d)
            if not use_f32r:
                acat = constp.tile([128, 128], f32)
                nc.sync.dma_start(acat[:], a_d)
            if with_bias:
                bias_t = constp.tile([128, 128], f32)
                nc.sync.dma_start(bias_t[:], bias_d)

            for blk in range(nblocks):
                xbig = xp.tile([128, NB * 128], mmdt)
                ybig = yp.tile([128, NB * 128], f32)
                for g in range(2):
                    nc.sync.dma_start(
                        xbig[g * 64 : (g + 1) * 64, :], x_d[g, blk]
                    )

                for grp in range(NB // GRP):
                    o1 = ps1.tile([128, GRP * W], f32)
                    o2 = ps2.tile([128, GRP * W], f32)
                    u = up.tile([128, GRP * 128], f32r if use_f32r else f32)
                    for q in range(GRP):
                        s = grp * GRP + q
                        nc.tensor.matmul(
                            o1[:, q * W : (q + 1) * W],
                            xbig[:, s * 128 : (s + 1) * 128],
                            wcat[:],
                        )
                    # useful half of o1 (cols [0,128) of each W-slot) -> u
                    if use_f32r:
                        o1u = o1[:].rearrange(
                            "p (q two f) -> p q two f", two=2, f=128
                        )[:, :, 0, :]
                    else:
                        o1u = o1[:].rearrange("p (q f) -> p q f", f=128)
                    nc.vector.tensor_copy(
                        u[:].rearrange("p (q f) -> p q f", f=128), o1u
                    )
                    for q in range(GRP):
                        nc.tensor.matmul(
                            o2[:, q * W : (q + 1) * W],
                            u[:, q * 128 : (q + 1) * 128],
                            wcat[:] if use_f32r else acat[:],
                        )
                    # useful half of o2 (right half of each W-slot for f32r)
                    if use_f32r:
                        o2u = o2[:].rearrange(
                            "p (q two f) -> p q two f", two=2, f=128
                        )[:, :, 1, :]
                    else:
                        o2u = o2[:].rearrange("p (q f) -> p q f", f=128)
                    ydst = ybig[:, grp * 512 : (grp + 1) * 512].rearrange(
                        "p (q f) -> p q f", f=128
                    )
                    nc.scalar.activation(
                        ydst, o2u, mybir.ActivationFunctionType.Gelu
                    )
                    if with_bias:
                        bseg = ybig[:, grp * 512 : (grp + 1) * 512].rearrange(
                            "p (q f) -> p q f", f=128
                        )
                        bsrc = bias_t[:].unsqueeze(1).broadcast_to([128, GRP, 128])
                        nc.vector.tensor_add(bseg, bseg, bsrc)

                for g in range(2):
                    nc.gpsimd.dma_start(
                        y_d[g, blk], ybig[g * 64 : (g + 1) * 64, :]
                    )

    nc.compile()
    return nc


def _get_nc(tpc, mm_impl, with_bias, n_cores=N_CORES):
    key = (tpc, mm_impl, with_bias, n_cores)
    if key not in _CACHE:
        _CACHE[key] = _build(*key)
    return _CACHE[key]


def _make_weights(A, B, mm_impl):
    Bd = np.zeros((128, 128), np.float32)
    Bd[:64, :64] = B
    Bd[64:, 64:] = B
    Ad = np.zeros((128, 128), np.float32)
    Ad[:64, :64] = A
    Ad[64:, 64:] = A
    if mm_impl == "f32r":
        return {"wcat": np.ascontiguousarray(np.concatenate([Bd, Ad], axis=1))}
    return {"wcat": Bd, "acat": Ad}


def _shard_perm(tpc):
    # per-core token order is identity: x rows [c*tpc:(c+1)*tpc] feed the
    # view (g h blk s); the view itself encodes t = g*tpc/2 + h*tpc/4 + s.
    return None


def _run(x, A, B, bias, mm_impl="f32r", tpc=TPC, trace=False):
    from concourse.bass_utils import run_bass_kernel_spmd

    n = x.shape[0]
    n_cores = n // tpc
    assert n == n_cores * tpc

    with_bias = bool(np.any(bias))
    nc = _get_nc(tpc, mm_impl, with_bias, n_cores)
    wmaps = _make_weights(np.asarray(A, np.float32), np.asarray(B, np.float32), mm_impl)

    nsuper = tpc // 4
    NB = min(32, nsuper)
    nblocks = nsuper // NB

    def permute_x(xs):
        # [g, blk, r, i, k] -> [g, blk, i, r, k], flattened per (g, blk)
        v = xs.reshape(2, nblocks, 2 * NB, 64, 64).transpose(0, 1, 3, 2, 4)
        return np.ascontiguousarray(v.reshape(2, nblocks, -1), dtype=np.float32)

    def unpermute_y(yd):
        # [g, blk, j, r, l] -> tokens x features
        v = yd.reshape(2, nblocks, 64, 2 * NB, 64).transpose(0, 1, 3, 2, 4)
        return v.reshape(tpc, D)

    in_maps = []
    for c in range(n_cores):
        m = {"x": permute_x(np.asarray(x[c * tpc : (c + 1) * tpc], dtype=np.float32))}
        m.update(wmaps)
        if with_bias:
            m["bias_t"] = np.ascontiguousarray(
                np.tile(bias.astype(np.float32).reshape(64, 64), (2, 2))
            )
        in_maps.append(m)

    res = run_bass_kernel_spmd(
        nc, in_maps, list(range(n_cores)), trace=trace,
        trace_cores=list(range(n_cores)) if trace else None,
    )
    y = np.concatenate([unpermute_y(r["y"]) for r in res.results], axis=0)
    return y.astype(np.float32), res


def kernel(x, A, B, bias):
    y, _ = _run(
        np.asarray(x), np.asarray(A), np.asarray(B), np.asarray(bias),
        mm_impl="f32r",
    )
    return y


# revision 14
# speedup vs baseline: 6.0048x; 1.0480x over previous
"""Kronecker layer forward on 8 TRN2 NeuronCores.

Computes y = gelu_exact(x @ kron(B, A)) + bias for
  x [16384, 4096] f32, A [64, 64], B [64, 64], bias [4096].

Math: with x3 = x.reshape(n, 64, 64) (feature f = i*64 + k),
  y[b, j*64+l] = sum_{i,k} x3[b,i,k] * B[i,j] * A[k,l].

Per supertile s we pick 4 tokens t(g,h) = g*(tpc/2) + h*(tpc/4) + s
(g,h in {0,1}) and form one 128x128 SBUF tile
  xt[(g,i), (h,k)] = x[t(g,h), i*64+k]
then chain two TensorE matmuls with the DATA as the stationary operand and
a 256-wide concatenated pair of block-diagonal factors as the moving one:
  o1 = xt.T @ [blockdiag(B,B) | blockdiag(A,A)]   left  half: [(h,k),(g,j)]
  o2 = u.T  @ [blockdiag(B,B) | blockdiag(A,A)]   right half: [(g,j),(h,l)]
(u = left half of o1 copied to SBUF). o2's right half is exactly the
y-layout view y[t(g,h), j*64+l] — both contraction dims land on partitions
with zero transposes. The 256-wide moving operand keeps float32r matmuls
at 1 cycle/row (4x faster than fp32) at the cost of one garbage half per
matmul.

The block-token mapping makes every DMA a single 3-dim access pattern:
one block = NB supertiles, loaded/stored with 4 big DMAs each (one per
(g,h) quadrant), 256B contiguous bursts.

Sharding: pure data-parallel over the token dim — 2048 tokens per core,
A/B/bias replicated, no collectives.
"""

import numpy as np

N_CORES = 8
TOKENS = 16384
D = 4096
TPC = TOKENS // N_CORES  # tokens per core

_CACHE = {}


def _build(tpc, mm_impl, with_bias, n_cores):
    import concourse.bacc as bacc
    import concourse.mybir as mybir
    import concourse.tile as tile

    f32 = mybir.dt.float32
    f32r = mybir.dt.float32r
    use_f32r = mm_impl == "f32r"

    nsuper = tpc // 4
    NB = min(16, nsuper)          # supertiles per block
    assert nsuper % NB == 0
    nblocks = nsuper // NB
    GRP = 4                       # supertiles per PSUM pack
    assert NB % GRP == 0
    W = 256 if use_f32r else 128  # matmul moving width / psum cols per st

    nc = bacc.Bacc(
        "TRN2",
        target_bir_lowering=False,
        debug=False,
        num_devices=n_cores,
    )
    mmdt = f32r if use_f32r else f32
    x_d = nc.dram_tensor(
        "x", [2, nblocks, 64 * 2 * NB * 64], mmdt, kind="ExternalInput"
    ).ap()
    w_d = nc.dram_tensor("wcat", [128, W], mmdt, kind="ExternalInput").ap()
    if not use_f32r:
        a_d = nc.dram_tensor("acat", [128, 128], f32, kind="ExternalInput").ap()
    if with_bias:
        bias_d = nc.dram_tensor("bias_t", [128, 128], f32, kind="ExternalInput").ap()
    y_d = nc.dram_tensor(
        "y", [2, nblocks, 64 * 2 * NB * 64], f32, kind="ExternalOutput"
    ).ap()

    # The host pre-permutes x to [g, blk, i, r, k] (r = 2s+h, token
    # t = g*tpc/2 + blk*2NB + r) so each (g, blk) slab is one fully
    # contiguous 1MB DMA whose element order matches the SBUF tile
    # [p=(g,i), free=r*64+k] exactly. y is written the same way
    # ([g, blk, j, r, l]) and inverse-permuted on the host.


    with tile.TileContext(nc) as tc:
        with (
            tc.tile_pool(name="const", bufs=1) as constp,
            tc.tile_pool(name="xp", bufs=4) as xp,
            tc.tile_pool(name="up", bufs=4) as up,
            tc.tile_pool(name="yp", bufs=4) as yp,
            tc.tile_pool(name="ps1", bufs=2, space="PSUM") as ps1,
            tc.tile_pool(name="ps2", bufs=2, space="PSUM") as ps2,
        ):
            wcat = constp.tile([128, W], mmdt)
            nc.sync.dma_start(wcat[:], w_d)
            if not use_f32r:
                acat = constp.tile([128, 128], f32)
                nc.sync.dma_start(acat[:], a_d)
            if with_bias:
                bias_t = constp.tile([128, 128], f32)
                nc.sync.dma_start(bias_t[:], bias_d)

            for blk in range(nblocks):
                xbig = xp.tile([128, NB * 128], mmdt)
                ybig = yp.tile([128, NB * 128], f32)
                for g in range(2):
                    nc.sync.dma_start(
                        xbig[g * 64 : (g + 1) * 64, :], x_d[g, blk]
                    )

                for grp in range(NB // GRP):
                    o1 = ps1.tile([128, GRP * W], f32)
                    o2 = ps2.tile([128, GRP * W], f32)
                    u = up.tile([128, GRP * 128], f32r if use_f32r else f32)
                    for q in range(GRP):
                        s = grp * GRP + q
                        nc.tensor.matmul(
                            o1[:, q * W : (q + 1) * W],
                            xbig[:, s * 128 : (s + 1) * 128],
                            wcat[:],
                        )
                    # useful half of o1 (cols [0,128) of each W-slot) -> u
                    if use_f32r:
                        o1u = o1[:].rearrange(
                            "p (q two f) -> p q two f", two=2, f=128
                        )[:, :, 0, :]
                    else:
                        o1u = o1[:].rearrange("p (q f) -> p q f", f=128)
                    nc.vector.tensor_copy(
                        u[:].rearrange("p (q f) -> p q f", f=128), o1u
                    )
                    for q in range(GRP):
                        nc.tensor.matmul(
                            o2[:, q * W : (q + 1) * W],
                            u[:, q * 128 : (q + 1) * 128],
                            wcat[:] if use_f32r else acat[:],
                        )
                    # useful half of o2 (right half of each W-slot for f32r)
                    if use_f32r:
                        o2u = o2[:].rearrange(
                            "p (q two f) -> p q two f", two=2, f=128
                        )[:, :, 1, :]
                    else:
                        o2u = o2[:].rearrange("p (q f) -> p q f", f=128)
                    ydst = ybig[:, grp * 512 : (grp + 1) * 512].rearrange(
                        "p (q f) -> p q f", f=128
                    )
                    nc.scalar.activation(
                        ydst, o2u, mybir.ActivationFunctionType.Gelu
                    )
                    if with_bias:
                        bseg = ybig[:, grp * 512 : (grp + 1) * 512].rearrange(
                            "p (q f) -> p q f", f=128
                        )
                        bsrc = bias_t[:].unsqueeze(1).broadcast_to([128, GRP, 128])
                        nc.vector.tensor_add(bseg, bseg, bsrc)

                for g in range(2):
                    nc.gpsimd.dma_start(
                        y_d[g, blk], ybig[g * 64 : (g + 1) * 64, :]
                    )

    nc.compile()
    return nc


def _get_nc(tpc, mm_impl, with_bias, n_cores=N_CORES):
    key = (tpc, mm_impl, with_bias, n_cores)
    if key not in _CACHE:
        _CACHE[key] = _build(*key)
    return _CACHE[key]


def _make_weights(A, B, mm_impl):
    Bd = np.zeros((128, 128), np.float32)
    Bd[:64, :64] = B
    Bd[64:, 64:] = B
    Ad = np.zeros((128, 128), np.float32)
    Ad[:64, :64] = A
    Ad[64:, 64:] = A
    if mm_impl == "f32r":
        return {"wcat": np.ascontiguousarray(np.concatenate([Bd, Ad], axis=1))}
    return {"wcat": Bd, "acat": Ad}


def _shard_perm(tpc):
    # per-core token order is identity: x rows [c*tpc:(c+1)*tpc] feed the
    # view (g h blk s); the view itself encodes t = g*tpc/2 + h*tpc/4 + s.
    return None


def _run(x, A, B, bias, mm_impl="f32r", tpc=TPC, trace=False):
    from concourse.bass_utils import run_bass_kernel_spmd

    n = x.shape[0]
    n_cores = n // tpc
    assert n == n_cores * tpc

    with_bias = bool(np.any(bias))
    nc = _get_nc(tpc, mm_impl, with_bias, n_cores)
    wmaps = _make_weights(np.asarray(A, np.float32), np.asarray(B, np.float32), mm_impl)

    nsuper = tpc // 4
    NB = min(16, nsuper)
    nblocks = nsuper // NB

    def permute_x(xs):
        # [g, blk, r, i, k] -> [g, blk, i, r, k], flattened per (g, blk)
        v = xs.reshape(2, nblocks, 2 * NB, 64, 64).transpose(0, 1, 3, 2, 4)
        return np.ascontiguousarray(v.reshape(2, nblocks, -1), dtype=np.float32)

    def unpermute_y(yd):
        # [g, blk, j, r, l] -> tokens x features
        v = yd.reshape(2, nblocks, 64, 2 * NB, 64).transpose(0, 1, 3, 2, 4)
        return v.reshape(tpc, D)

    in_maps = []
    for c in range(n_cores):
        m = {"x": permute_x(np.asarray(x[c * tpc : (c + 1) * tpc], dtype=np.float32))}
        m.update(wmaps)
        if with_bias:
            m["bias_t"] = np.ascontiguousarray(
                np.tile(bias.astype(np.float32).reshape(64, 64), (2, 2))
            )
        in_maps.append(m)

    res = run_bass_kernel_spmd(
        nc, in_maps, list(range(n_cores)), trace=trace,
        trace_cores=list(range(n_cores)) if trace else None,
    )
    y = np.concatenate([unpermute_y(r["y"]) for r in res.results], axis=0)
    return y.astype(np.float32), res


def kernel(x, A, B, bias):
    y, _ = _run(
        np.asarray(x), np.asarray(A), np.asarray(B), np.asarray(bias),
        mm_impl="f32r",
    )
    return y
